# revision 6
# baseline (speedup 1.0000x reference)
"""ALiBi bias add: out[b,h,q,k] = x[b,h,q,k] + slope[h] * (k - (S-1)).

Input (1, 16, 4096, 4096) f32, sharded 2 heads per core across 8 NeuronCores.
Per core: stream (128, R*4096) tiles HBM -> SBUF, one fused DVE op
    out = dist * slope_h + x   (scalar_tensor_tensor, slope as [P,1] operand)
with the distance row built once on-chip via iota, then DMA back.
Memory-bound: 268 MB HBM traffic per core.
"""

import numpy as np

import concourse.bacc as bacc
import concourse.bass as bass
import concourse.mybir as mybir
from concourse import tile
from concourse.bass_utils import run_bass_kernel_spmd

N_CORES = 8
H = 16
S = 4096
H_LOC = H // N_CORES          # heads per core
R = 2                         # query rows per partition per tile
F = R * S                     # free-dim elements per tile
NT = S // (128 * R)           # tiles per head
F32 = mybir.dt.float32


def _alibi_slopes(n_heads: int) -> np.ndarray:
    def _slopes_with_step(n, step=1):
        ratio = 2.0 ** (-8.0 / n)
        return ratio ** np.arange(1, 1 + n, step, dtype=np.float64)

    k = 1 << (n_heads.bit_length() - 1)
    slopes = _slopes_with_step(k)
    if n_heads != k:
        remaining = n_heads - k
        slopes = np.concatenate([slopes, _slopes_with_step(2 * k, step=2)[:remaining]])
    return slopes.astype(np.float32)


def build_nc(data_bufs: int = 4) -> bass.Bass:
    # Bacc (not raw Bass): its compile() runs generate_event_semaphores,
    # which splits multi-sem waits to satisfy TRN2's 1-wait-per-instruction
    # constraint.
    nc = bacc.Bacc()
    x = nc.declare_dram_parameter("x", [H_LOC * S, S], F32, isOutput=False)
    slopes = nc.declare_dram_parameter("slopes", [1, H_LOC], F32, isOutput=False)
    out = nc.declare_dram_parameter("out", [H_LOC * S, S], F32, isOutput=True)

    with tile.TileContext(nc) as tc:
        with (
            tc.tile_pool(name="const", bufs=1) as cpool,
            tc.tile_pool(name="data", bufs=data_bufs) as dpool,
        ):
            # slopes replicated to all 128 partitions: (128, H_LOC)
            slope_t = cpool.tile([128, H_LOC], F32)
            nc.sync.dma_start(
                out=slope_t[:, :],
                in_=slopes[:, :].partition_broadcast(128).squeeze(1),
            )
            # distance row, replicated R times along free dim: k - (S-1)
            dist_t = cpool.tile([128, F], F32)
            nc.gpsimd.iota(
                dist_t[:, :].rearrange("p (r m) -> p r m", r=R),
                pattern=[[0, R], [1, S]],
                base=-(S - 1),
                channel_multiplier=0,
                allow_small_or_imprecise_dtypes=True,
            )

            # Absorb the slope-DMA and iota completion sems onto DVE via two
            # tiny single-dep ops (the STT encoding has one sync-wait slot,
            # so no later instruction may need >1 semaphore wait).
            scratch = cpool.tile([128, 1], F32)
            nc.vector.tensor_scalar_add(scratch[:, :], slope_t[:, 0:1], 0.0)
            nc.vector.tensor_scalar_add(scratch[:, :], dist_t[:, 0:1], 0.0)

            xv = x[:, :].rearrange("(h n p r) m -> h n p (r m)", h=H_LOC, p=128, r=R)
            ov = out[:, :].rearrange("(h n p r) m -> h n p (r m)", h=H_LOC, p=128, r=R)
            for h in range(H_LOC):
                for n in range(NT):
                    t = dpool.tile([128, F], F32, tag="data")
                    nc.sync.dma_start(out=t[:, :], in_=xv[h, n])
                    nc.vector.scalar_tensor_tensor(
                        out=t[:, :],
                        in0=dist_t[:, :],
                        scalar=slope_t[:, h : h + 1],
                        in1=t[:, :],
                        op0=mybir.AluOpType.mult,
                        op1=mybir.AluOpType.add,
                    )
                    nc.scalar.dma_start(out=ov[h, n], in_=t[:, :])
    nc.finalize()
    return nc


_NC_CACHE: bass.Bass | None = None


def _get_nc() -> bass.Bass:
    global _NC_CACHE
    if _NC_CACHE is None:
        _NC_CACHE = build_nc()
    return _NC_CACHE


def _make_in_maps(x: np.ndarray) -> list[dict[str, np.ndarray]]:
    x = np.ascontiguousarray(np.asarray(x, dtype=np.float32)).reshape(H, S, S)
    slopes = _alibi_slopes(H)
    in_maps = []
    for c in range(N_CORES):
        shard = x[c * H_LOC : (c + 1) * H_LOC].reshape(H_LOC * S, S)
        sl = np.ascontiguousarray(slopes[c * H_LOC : (c + 1) * H_LOC]).reshape(
            1, H_LOC
        )
        in_maps.append({"x": shard, "slopes": sl})
    return in_maps


def kernel(attention_scores: np.ndarray, _trace: bool = False, **_tr_kwargs):
    nc = _get_nc()
    in_maps = _make_in_maps(attention_scores)
    res = run_bass_kernel_spmd(
        nc, in_maps, core_ids=list(range(N_CORES)), trace=_trace, **_tr_kwargs
    )
    outs = [np.asarray(res.results[i]["out"]).reshape(H_LOC, S, S) for i in range(N_CORES)]
    full = np.concatenate(outs, axis=0).reshape(1, H, S, S)
    if _trace:
        return full, res
    return full
